# revision 5
# baseline (speedup 1.0000x reference)
"""ConvMod kernel for Trainium2 (8 NeuronCores, batch-parallel).

Per-sample modulated 3x3 grouped conv:
  style = w @ (fce_kernel*fce_scale) + fce_bias                [B, CIN]
  wp    = conv_kernel * conv_scale * style                     [B,3,3,CIN,NF]
  wpp   = wp * rsqrt(sum(wp^2, (ky,kx,cin)) + 1e-8)            demodulated
  out   = conv2d_same(x, wpp per-sample) + conv_bias           [B,H,W,NF]

Sharding: batch B=8 across 8 cores (1 sample/core), params replicated.
Device layout: channels on partitions; host transposes x to [C,H,W] per
sample and the [NF, H*W] device output back to NHWC.
"""

import numpy as np

B, H, W, CIN = 8, 256, 256, 64
WDIM, NF, KK = 512, 64, 3
NCORES = 8
CR = 32  # output rows per x chunk
FCE_SCALE = float(np.sqrt(1.0 / WDIM))
CONV_SCALE = float(np.sqrt(1.0 / 0.6 / (KK * KK * CIN)))

_CACHE = {}


def _build(repeats=1):
    import concourse.mybir as mybir
    import concourse.tile as tile
    from concourse import bacc

    f32 = mybir.dt.float32
    f32r = mybir.dt.float32r
    nc = bacc.Bacc("TRN2", target_bir_lowering=False, debug=False,
                   num_devices=NCORES)

    xt = nc.dram_tensor("xt", [CIN, H, W], f32r, kind="ExternalInput").ap()
    wv = nc.dram_tensor("wv", [WDIM], f32, kind="ExternalInput").ap()
    fce_k = nc.dram_tensor("fce_k", [WDIM, CIN], f32, kind="ExternalInput").ap()
    fce_b = nc.dram_tensor("fce_b", [CIN], f32, kind="ExternalInput").ap()
    ck_d = nc.dram_tensor("ck", [KK, KK, CIN, NF], f32, kind="ExternalInput").ap()
    cb_d = nc.dram_tensor("cb", [NF], f32, kind="ExternalInput").ap()
    zpad = nc.dram_tensor("zpad", [CIN, W + 2], f32r, kind="ExternalInput").ap()
    yt = nc.dram_tensor("yt", [NF, H * W], f32, kind="ExternalOutput").ap()

    NT = KK * KK  # 9 taps
    with tile.TileContext(nc) as tc:
        with (
            tc.tile_pool(name="const", bufs=1) as const,
            tc.tile_pool(name="prep", bufs=1) as prep,
            tc.tile_pool(name="pps", bufs=1, space="PSUM") as pps,
            tc.tile_pool(name="xin", bufs=2) as xin,
            tc.tile_pool(name="yout", bufs=3) as yout,
            tc.tile_pool(name="acc", bufs=4, space="PSUM") as accp,
        ):
            # ---- weight prep (tiny) ----
            fce_sb = prep.tile([128, WDIM // 128, CIN], f32)
            nc.sync.dma_start(out=fce_sb,
                              in_=fce_k.rearrange("(j p) c -> p j c", p=128))
            wv_sb = prep.tile([128, WDIM // 128], f32)
            nc.sync.dma_start(out=wv_sb,
                              in_=wv.rearrange("(j p) -> p j", p=128))
            fce_b_sb = prep.tile([CIN, 1], f32)
            nc.sync.dma_start(out=fce_b_sb, in_=fce_b)
            ck_sb = prep.tile([CIN, NT, NF], f32)
            nc.sync.dma_start(out=ck_sb,
                              in_=ck_d.rearrange("ky kx c n -> c ky kx n"))
            cb_sb = const.tile([NF, 1], f32)
            nc.sync.dma_start(out=cb_sb, in_=cb_d)

            ones_k = const.tile([CIN, 1], f32)
            nc.vector.memset(ones_k, 1.0)
            ones_m = const.tile([1, NF], f32)
            nc.vector.memset(ones_m, 1.0)

            # style = w @ (fce_k * fce_scale) + fce_b, then * conv_scale
            style_ps = pps.tile([CIN, 1], f32)
            for j in range(WDIM // 128):
                nc.tensor.matmul(style_ps, lhsT=fce_sb[:, j, :],
                                 rhs=wv_sb[:, j:j + 1],
                                 start=(j == 0), stop=(j == WDIM // 128 - 1))
            fce_b_sc = prep.tile([CIN, 1], f32)
            nc.scalar.mul(out=fce_b_sc, in_=fce_b_sb, mul=CONV_SCALE)
            stylec = prep.tile([CIN, 1], f32)
            nc.scalar.activation(stylec, style_ps,
                                 mybir.ActivationFunctionType.Identity,
                                 bias=fce_b_sc, scale=FCE_SCALE * CONV_SCALE)

            # wp[c, t, n] = ck * stylec[c];  sq = wp^2
            wp = prep.tile([CIN, NT, NF], f32)
            nc.vector.tensor_scalar_mul(wp.rearrange("c t n -> c (t n)"),
                                        ck_sb.rearrange("c t n -> c (t n)"),
                                        stylec)
            sq = prep.tile([CIN, NT, NF], f32)
            nc.vector.tensor_mul(sq.rearrange("c t n -> c (t n)"),
                                 wp.rearrange("c t n -> c (t n)"),
                                 wp.rearrange("c t n -> c (t n)"))
            tap_acc = prep.tile([CIN, NF], f32)
            nc.vector.tensor_add(tap_acc, sq[:, 0, :], sq[:, 1, :])
            for t in range(2, NT):
                nc.vector.tensor_add(tap_acc, tap_acc, sq[:, t, :])
            ssum_ps = pps.tile([1, NF], f32)
            nc.tensor.matmul(ssum_ps, lhsT=ones_k, rhs=tap_acc,
                             start=True, stop=True)
            eps_sb = prep.tile([1, 1], f32)
            nc.vector.memset(eps_sb, 1e-8)
            sroot = prep.tile([1, NF], f32)
            nc.scalar.activation(sroot, ssum_ps,
                                 mybir.ActivationFunctionType.Sqrt,
                                 bias=eps_sb, scale=1.0)
            wstd = prep.tile([1, NF], f32)
            nc.vector.reciprocal(wstd, sroot)
            bcast_ps = pps.tile([CIN, NF], f32)
            nc.tensor.matmul(bcast_ps, lhsT=ones_m, rhs=wstd,
                             start=True, stop=True)
            wsb = const.tile([CIN, NT, NF], f32r)
            for t in range(NT):
                nc.vector.tensor_mul(wsb[:, t, :], wp[:, t, :], bcast_ps)

            # ---- main conv loop ----
            NCH = H // CR           # chunks
            GPC = CR // 2           # row-pair groups per chunk
            for _ in range(repeats):
                for ci in range(NCH):
                    xc = xin.tile([CIN, CR + 2, W + 2], f32r)
                    r0 = ci * CR - 1
                    l0 = 1 if ci == 0 else 0
                    l1 = (CR + 1) if ci == NCH - 1 else (CR + 2)
                    nc.sync.dma_start(
                        out=xc[:, l0:l1, 1:W + 1],
                        in_=xt[:, max(r0, 0):min(r0 + CR + 2, H), :])
                    nc.sync.dma_start(out=xc[:, :, 0:1], in_=zpad[:, 0:CR + 2])
                    nc.sync.dma_start(out=xc[:, :, W + 1:W + 2],
                                      in_=zpad[:, 0:CR + 2])
                    if ci == 0:
                        nc.sync.dma_start(out=xc[:, 0:1, :], in_=zpad)
                    if ci == NCH - 1:
                        nc.sync.dma_start(out=xc[:, CR + 1:CR + 2, :], in_=zpad)

                    for gi in range(GPC):
                        g = ci * GPC + gi
                        s = g % 4
                        if s == 0:
                            ys = yout.tile([NF, 4, 2 * W], f32)
                        ps = accp.tile([NF, 2 * W], f32)
                        for t in range(NT):
                            dy, dx = t // 3 - 1, t % 3 - 1
                            lrow = gi * 2 + dy + 1
                            rhs = xc[:, lrow:lrow + 2, 1 + dx:1 + dx + W]
                            nc.tensor.matmul(ps, lhsT=wsb[:, t, :],
                                             rhs=rhs,
                                             start=(t == 0), stop=(t == NT - 1))
                        nc.scalar.activation(ys[:, s, :], ps,
                                             mybir.ActivationFunctionType.Identity,
                                             bias=cb_sb, scale=1.0)
                        if s == 3:
                            nc.sync.dma_start(
                                out=yt[:, (g - 3) * 2 * W:(g + 1) * 2 * W],
                                in_=ys)

    nc.compile()
    return nc


def _get(repeats=1):
    if repeats not in _CACHE:
        _CACHE[repeats] = _build(repeats)
    return _CACHE[repeats]


def kernel(x, w, fce_kernel, fce_bias, conv_kernel, conv_bias):
    from concourse.bass_utils import run_bass_kernel_spmd

    nc = _get()
    in_maps = []
    for b in range(B):
        in_maps.append({
            "xt": np.ascontiguousarray(np.asarray(x[b], np.float32).transpose(2, 0, 1)),
            "wv": np.ascontiguousarray(np.asarray(w[b], np.float32)),
            "fce_k": np.asarray(fce_kernel, np.float32),
            "fce_b": np.asarray(fce_bias, np.float32),
            "ck": np.asarray(conv_kernel, np.float32),
            "cb": np.asarray(conv_bias, np.float32),
            "zpad": np.zeros((CIN, W + 2), np.float32),
        })
    res = run_bass_kernel_spmd(nc, in_maps, core_ids=list(range(NCORES)))
    out = np.empty((B, H, W, NF), np.float32)
    for b in range(B):
        out[b] = res.results[b]["yt"].reshape(NF, H, W).transpose(1, 2, 0)
    return out


# revision 20
# speedup vs baseline: 438.1543x; 438.1543x over previous
"""ConvMod kernel for Trainium2 (8 NeuronCores, batch-parallel).

Per-sample modulated 3x3 grouped conv:
  style = w @ (fce_kernel*fce_scale) + fce_bias                [B, CIN]
  wp    = conv_kernel * conv_scale * style                     [B,3,3,CIN,NF]
  wpp   = wp * rsqrt(sum(wp^2, (ky,kx,cin)) + 1e-8)            demodulated
  out   = conv2d_same(x, wpp per-sample) + conv_bias           [B,H,W,NF]

Sharding: batch B=8 across 8 cores (1 sample/core), params replicated.
Device layout: channels on partitions; host transposes x to [C,H,W] per
sample and the [NF, H*W] device output back to NHWC.
"""

import numpy as np

B, H, W, CIN = 8, 256, 256, 64
WDIM, NF, KK = 512, 64, 3
NCORES = 8
CR = 32  # output rows per x chunk
FCE_SCALE = float(np.sqrt(1.0 / WDIM))
CONV_SCALE = float(np.sqrt(1.0 / 0.6 / (KK * KK * CIN)))

_CACHE = {}


def _build(repeats=1):
    import os as _os
    _skip_mm = _os.environ.get('KSKIP_MM') == '1'
    _skip_dup = _os.environ.get('KSKIP_DUP') == '1'
    import concourse.bass as bass_mod
    import concourse.mybir as mybir
    import concourse.tile as tile
    from concourse import bacc

    f32 = mybir.dt.float32
    f32r = mybir.dt.float32r
    nc = bacc.Bacc("TRN2", target_bir_lowering=False, debug=False,
                   num_devices=NCORES)

    xt = nc.dram_tensor("xt", [CIN, H, W], f32r, kind="ExternalInput").ap()
    wv = nc.dram_tensor("wv", [WDIM], f32, kind="ExternalInput").ap()
    fce_k = nc.dram_tensor("fce_k", [WDIM, CIN], f32, kind="ExternalInput").ap()
    fce_b = nc.dram_tensor("fce_b", [CIN], f32, kind="ExternalInput").ap()
    ck_d = nc.dram_tensor("ck", [KK, KK, CIN, NF], f32, kind="ExternalInput").ap()
    cb_d = nc.dram_tensor("cb", [NF], f32, kind="ExternalInput").ap()
    zpad = nc.dram_tensor("zpad", [CIN, W + 2], f32r, kind="ExternalInput").ap()
    yt = nc.dram_tensor("yt", [NF, H * W], f32, kind="ExternalOutput").ap()

    NT = KK * KK  # 9 taps
    with tile.TileContext(nc) as tc:
        with (
            tc.tile_pool(name="const", bufs=1) as const,
            tc.tile_pool(name="prep", bufs=1) as prep,
            tc.tile_pool(name="pps", bufs=1, space="PSUM") as pps,
            tc.tile_pool(name="xin", bufs=3) as xin,
            tc.tile_pool(name="yout", bufs=4) as yout,
            tc.tile_pool(name="acc", bufs=5, space="PSUM") as accp,
        ):
            # ---- weight prep (tiny) ----
            fce_sb = prep.tile([128, WDIM // 128, CIN], f32)
            nc.sync.dma_start(out=fce_sb,
                              in_=fce_k.rearrange("(j p) c -> p j c", p=128))
            wv_sb = prep.tile([128, WDIM // 128], f32)
            nc.sync.dma_start(out=wv_sb,
                              in_=wv.rearrange("(j p) -> p j", p=128))
            fce_b_sb = prep.tile([CIN, 1], f32)
            nc.sync.dma_start(out=fce_b_sb, in_=fce_b)
            ck_sb = prep.tile([CIN, NT, NF], f32)
            nc.sync.dma_start(out=ck_sb,
                              in_=ck_d.rearrange("ky kx c n -> c ky kx n"))
            cb_sb = const.tile([NF, 1], f32)
            nc.sync.dma_start(out=cb_sb, in_=cb_d)
            # conv_bias replicated on partitions 64-127 for col-tiled pairs
            cb2_sb = const.tile([2 * NF, 1], f32)
            nc.sync.dma_start(out=cb2_sb[0:NF, :], in_=cb_d)
            nc.sync.dma_start(out=cb2_sb[NF:2 * NF, :], in_=cb_d)

            ones_k = const.tile([CIN, 1], f32)
            nc.vector.memset(ones_k, 1.0)
            ones_m = const.tile([1, NF], f32)
            nc.vector.memset(ones_m, 1.0)

            # style = w @ (fce_k * fce_scale) + fce_b, then * conv_scale
            style_ps = pps.tile([CIN, 1], f32)
            for j in range(WDIM // 128):
                nc.tensor.matmul(style_ps, lhsT=fce_sb[:, j, :],
                                 rhs=wv_sb[:, j:j + 1],
                                 start=(j == 0), stop=(j == WDIM // 128 - 1))
            fce_b_sc = prep.tile([CIN, 1], f32)
            nc.scalar.mul(out=fce_b_sc, in_=fce_b_sb, mul=CONV_SCALE)
            stylec = prep.tile([CIN, 1], f32)
            nc.scalar.activation(stylec, style_ps,
                                 mybir.ActivationFunctionType.Identity,
                                 bias=fce_b_sc, scale=FCE_SCALE * CONV_SCALE)

            # wp[c, t, n] = ck * stylec[c];  sq = wp^2
            wp = prep.tile([CIN, NT, NF], f32)
            nc.vector.tensor_scalar_mul(wp.rearrange("c t n -> c (t n)"),
                                        ck_sb.rearrange("c t n -> c (t n)"),
                                        stylec)
            sq = prep.tile([CIN, NT, NF], f32)
            nc.vector.tensor_mul(sq.rearrange("c t n -> c (t n)"),
                                 wp.rearrange("c t n -> c (t n)"),
                                 wp.rearrange("c t n -> c (t n)"))
            tap_acc = prep.tile([CIN, NF], f32)
            nc.vector.tensor_add(tap_acc, sq[:, 0, :], sq[:, 1, :])
            for t in range(2, NT):
                nc.vector.tensor_add(tap_acc, tap_acc, sq[:, t, :])
            ssum_ps = pps.tile([1, NF], f32)
            nc.tensor.matmul(ssum_ps, lhsT=ones_k, rhs=tap_acc,
                             start=True, stop=True)
            eps_sb = prep.tile([1, 1], f32)
            nc.vector.memset(eps_sb, 1e-8)
            sroot = prep.tile([1, NF], f32)
            nc.scalar.activation(sroot, ssum_ps,
                                 mybir.ActivationFunctionType.Sqrt,
                                 bias=eps_sb, scale=1.0)
            wstd = prep.tile([1, NF], f32)
            nc.vector.reciprocal(wstd, sroot)
            bcast_ps = pps.tile([CIN, NF], f32)
            nc.tensor.matmul(bcast_ps, lhsT=ones_m, rhs=wstd,
                             start=True, stop=True)
            wsb = const.tile([CIN, NT, NF], f32r)
            for t in range(NT):
                nc.vector.tensor_mul(wsb[:, t, :], wp[:, t, :], bcast_ps)

            # stacked weights for K-packed tap pairs: partitions 0-63 hold
            # tap (dy,-1), partitions 64-127 hold tap (dy,0); a single K=128
            # matmul then contracts both taps at once (rhs band B = x
            # shifted left one column).
            w2 = const.tile([2 * CIN, KK, NF], f32r)
            for dyi in range(KK):
                nc.sync.dma_start(out=w2[0:CIN, dyi, :],
                                  in_=wsb[:, dyi * 3 + 0, :])
                nc.sync.dma_start(out=w2[CIN:2 * CIN, dyi, :],
                                  in_=wsb[:, dyi * 3 + 1, :])

            # ---- main conv loop ----
            NCH = H // CR           # chunks
            GPC = CR // 2           # row-pair groups per chunk
            for _ in range(repeats):
                for ci in range(NCH):
                    xc = xin.tile([2 * CIN, CR + 2, W + 2], f32r)
                    r0 = ci * CR - 1
                    l0 = 1 if ci == 0 else 0
                    l1 = (CR + 1) if ci == NCH - 1 else (CR + 2)
                    nc.sync.dma_start(
                        out=xc[0:CIN, l0:l1, 1:W + 1],
                        in_=xt[:, max(r0, 0):min(r0 + CR + 2, H), :])
                    nc.sync.dma_start(out=xc[0:CIN, :, 0:1],
                                      in_=zpad[:, 0:CR + 2])
                    nc.sync.dma_start(out=xc[0:CIN, :, W + 1:W + 2],
                                      in_=zpad[:, 0:CR + 2])
                    if ci == 0:
                        nc.sync.dma_start(out=xc[0:CIN, 0:1, :], in_=zpad)
                    if ci == NCH - 1:
                        nc.sync.dma_start(out=xc[0:CIN, CR + 1:CR + 2, :],
                                          in_=zpad)
                    # band B: same rows, shifted left one column
                    if not _skip_dup:
                        # split into row bands so early matmuls overlap the copy
                        nb = 4
                        rb = (CR + 2 + nb - 1) // nb
                        for bi in range(nb):
                            a, b = bi * rb, min((bi + 1) * rb, CR + 2)
                            if a >= b:
                                continue
                            nc.vector.tensor_copy(
                                xc[CIN:2 * CIN, a:b, 0:W + 1],
                                xc[0:CIN, a:b, 1:W + 2])

                    for gi in range(GPC):
                        g = ci * GPC + gi
                        s = g % 4
                        if s == 0:
                            ys = yout.tile([NF, 4, 2 * W], f32)
                        ps = accp.tile([NF, 2 * W], f32)
                        if not _skip_mm:
                            for dyi in range(KK):
                                lrow = gi * 2 + dyi
                                rhs = xc[:, lrow:lrow + 2, 0:W]
                                nc.tensor.matmul(ps, lhsT=w2[:, dyi, :], rhs=rhs,
                                                 start=(dyi == 0), stop=False)
                            for dyi in range(KK):
                                lrow = gi * 2 + dyi
                                rhs = xc[0:CIN, lrow:lrow + 2, 2:2 + W]
                                nc.tensor.matmul(ps, lhsT=wsb[:, dyi * 3 + 2, :],
                                                 rhs=rhs, start=False,
                                                 stop=(dyi == KK - 1))
                        else:
                            nc.tensor.matmul(ps, lhsT=wsb[:, 0, :],
                                             rhs=xc[0:CIN, 0:2, 0:W],
                                             start=True, stop=True)
                        nc.scalar.activation(ys[:, s, :], ps,
                                             mybir.ActivationFunctionType.Identity,
                                             bias=cb_sb, scale=1.0)
                        if s == 3:
                            nc.sync.dma_start(
                                out=yt[:, (g - 3) * 2 * W:(g + 1) * 2 * W],
                                in_=ys)

    nc.compile()
    return nc


def _get(repeats=1):
    if repeats not in _CACHE:
        _CACHE[repeats] = _build(repeats)
    return _CACHE[repeats]


def kernel(x, w, fce_kernel, fce_bias, conv_kernel, conv_bias):
    from concourse.bass_utils import run_bass_kernel_spmd

    nc = _get()
    in_maps = []
    for b in range(B):
        in_maps.append({
            "xt": np.ascontiguousarray(np.asarray(x[b], np.float32).transpose(2, 0, 1)),
            "wv": np.ascontiguousarray(np.asarray(w[b], np.float32)),
            "fce_k": np.asarray(fce_kernel, np.float32),
            "fce_b": np.asarray(fce_bias, np.float32),
            "ck": np.asarray(conv_kernel, np.float32),
            "cb": np.asarray(conv_bias, np.float32),
            "zpad": np.zeros((CIN, W + 2), np.float32),
        })
    res = run_bass_kernel_spmd(nc, in_maps, core_ids=list(range(NCORES)))
    out = np.empty((B, H, W, NF), np.float32)
    for b in range(B):
        out[b] = res.results[b]["yt"].reshape(NF, H, W).transpose(1, 2, 0)
    return out
